# revision 3
# baseline (speedup 1.0000x reference)
"""CGCNN v3: 3x CGConv + graph LayerNorm + global mean pool + MLP on 8 TRN2 cores.

Changes vs v2:
  - Custom ACT table order: the exp+ln set is first, so exp and ln resolve to
    one resident table (no per-group ACT_TABLE_LOAD thrash).
  - Src table stores EXPONENTIATED projections e^(-Fsrc)|e^(Ssrc); the per-tile
    identity-inject matmul becomes one batched DVE mult per (block, half) on
    the exp-domain values. 3 PE matmuls per tile (eproj, dst-inject, scatter).
  - Gather chunks = one per (block, half), aligned with tile runs.
  - One-hots built per block: ohr one DVE op; ohrT via a DMA partition-
    broadcast of the dloc row + one DVE compare.
  - Block-level epilogue; scatter delayed one block.
"""

import os

import numpy as np
import ml_dtypes

import concourse.bass as bass
import concourse.bacc as bacc
import concourse.mybir as mybir
import concourse.tile as tile
from concourse.bass_utils import run_bass_kernel_spmd
from concourse.library_config import mlp as _mlp_lib

BF16 = ml_dtypes.bfloat16
NC_CORES = 8


def _install_act_tables():
    if os.environ.get("CG_NO_ACTFIX"):
        return
    """Reorder act_func_sets so the exp+ln set is first: both Exp and Ln then
    resolve to one resident table and the per-switch 1.28us table reloads
    disappear. Both the bass set-id pass and walrus read the same file."""
    import json
    try:
        from neuronxcc.driver.Job import Job
        import neuronxcc.driver.jobs.support.FindActInfo as FAI
    except ImportError:
        return
    out = "/tmp/cg_act_info/act_info.json"
    if "CG_ACT_DONE" not in os.environ:
        import shutil
        try:
            srcf = FAI.findActInfoFile(Job.getPackageDir(), "gen3")
        except Exception:
            return
        info = json.load(open(srcf))
        sets = info["act_func_sets"]
        k = next((i for i, s in enumerate(sets)
                  if s["name"] == "natural_log_exp_and_others"), None)
        if k is None:
            return
        sets.insert(0, sets.pop(k))
        # the sets reference sibling *_bkt.bin/*_ctrl.bin table files
        shutil.rmtree("/tmp/cg_act_info", ignore_errors=True)
        shutil.copytree(os.path.dirname(srcf), "/tmp/cg_act_info")
        json.dump(info, open(out, "w"))
        os.environ["CG_ACT_DONE"] = "1"
    os.environ["BASS_ACT_ROOT_JSON_PATH"] = out
    FAI.findActInfoFile = lambda pkg, arch, _out=out: _out


_install_act_tables()
BLK = 128
GCHUNK = 1024
EPS = 1e-5

F32 = mybir.dt.float32
BF = mybir.dt.bfloat16
F8 = mybir.dt.float8e4
FP8 = ml_dtypes.float8_e4m3
I16 = mybir.dt.int16
AF = mybir.ActivationFunctionType
OP = mybir.AluOpType


def _ceil_div(a, b):
    return (a + b - 1) // b


def _wrap_idx16(idx):
    """[L] -> [128, L//16] in the dma_gather wrapped layout."""
    cols = len(idx) // 16
    w = idx.reshape(cols, 16).T
    return np.tile(w, (8, 1)).astype(np.int16)


def host_prep(x, edge_index, edge_attr, batch, W1, b1, layer_params, W2, b2, W3, b3):
    N, Din = x.shape
    G = 128
    D = 128
    De = edge_attr.shape[1]
    NP = N // NC_CORES
    NPP = _ceil_div(NP, BLK) * BLK
    NBLK = NPP // BLK
    HALF = 4 * NPP

    src = np.asarray(edge_index[0], np.int64)
    dst = np.asarray(edge_index[1], np.int64)
    batch = np.asarray(batch, np.int64)
    x = np.asarray(x, np.float32)
    edge_attr = np.asarray(edge_attr, np.float32)

    src_pad = (src // NP) * NPP + (src % NP)
    half_of_src = (src_pad >= HALF).astype(np.int64)

    # partition edges by dst owner; group per (block, half); sort by src
    per_core = []
    counts = np.zeros((NC_CORES, NBLK, 2), np.int64)
    for c in range(NC_CORES):
        m = (dst // NP) == c
        e_ids = np.nonzero(m)[0]
        dl = dst[e_ids] - c * NP
        b = dl // BLK
        h = half_of_src[e_ids]
        order = np.lexsort((src_pad[e_ids], h, b))
        e_ids, dl, b, h = e_ids[order], dl[order], b[order], h[order]
        per_core.append((e_ids, dl, b, h))
        for bb in range(NBLK):
            mb = b == bb
            counts[c, bb, 0] = np.count_nonzero(mb & (h == 0))
            counts[c, bb, 1] = np.count_nonzero(mb & (h == 1))

    ntiles = _ceil_div(np.max(counts, axis=0), BLK)  # [NBLK, 2]
    kAs = ntiles[:, 0].astype(int).tolist()
    kBs = ntiles[:, 1].astype(int).tolist()
    TOT = int(ntiles.sum())
    SLOTS = TOT * BLK
    bt0 = []   # first tile index of each block
    acc = 0
    for bb in range(NBLK):
        bt0.append(acc)
        acc += kAs[bb] + kBs[bb]
    t2b = []
    t2h = []
    for bb in range(NBLK):
        t2b += [bb] * (kAs[bb] + kBs[bb])
        t2h += [0] * kAs[bb] + [1] * kBs[bb]
    KT = max(kAs[bb] + kBs[bb] for bb in range(NBLK))
    KTA = max(kAs)
    KTB = max(kBs)
    assert KT * BLK < 2048, "per-(block,half) gather exceeds Q7 scratch"
    # idx column offsets per (block, half) in the packed idx tile
    cA = []
    cB = []
    colp = 0
    for bb in range(NBLK):
        cA.append(colp)
        colp += kAs[bb] * 8
        cB.append(colp)
        colp += kBs[bb] * 8
    ICOLS = colp

    sched = dict(N=N, G=G, D=D, De=De, Din=Din, NP=NP, NPP=NPP, NBLK=NBLK,
                 HALF=HALF, TOT=TOT, t2b=t2b, t2h=t2h,
                 kAs=kAs, kBs=kBs, bt0=bt0, KT=KT, KTA=KTA, KTB=KTB,
                 cA=cA, cB=cB, ICOLS=ICOLS)

    cnts = np.bincount(batch, minlength=G).astype(np.float32)
    inv_cnt = (1.0 / np.maximum(cnts, 1.0)).astype(np.float32)

    in_maps = []
    for c in range(NC_CORES):
        e_ids, dl, b, h = per_core[c]
        sidx = np.zeros(SLOTS, np.int64)
        dloc = np.full(SLOTS, -1.0, np.float32)
        eperm = np.zeros(SLOTS, np.int64) - 1
        ptr = 0
        for t in range(TOT):
            bb, hh = t2b[t], t2h[t]
            e0 = ptr
            while ptr < len(b) and b[ptr] == bb and h[ptr] == hh and ptr - e0 < BLK:
                ptr += 1
            cnt = ptr - e0
            s0 = t * BLK
            if cnt:
                ids = e_ids[e0:ptr]
                sidx[s0:s0 + cnt] = src_pad[ids]
                dloc[s0:s0 + cnt] = (dl[e0:ptr] % BLK).astype(np.float32)
                eperm[s0:s0 + cnt] = ids
        assert ptr == len(b)

        idx_all = np.zeros((128, ICOLS), np.int16)
        for bb in range(NBLK):
            tA = bt0[bb]
            kA, kB = kAs[bb], kBs[bb]
            if kA:
                v = np.maximum(sidx[tA * BLK:(tA + kA) * BLK], 0)
                idx_all[:, cA[bb]:cA[bb] + kA * 8] = _wrap_idx16(v)
            if kB:
                s0 = (tA + kA) * BLK
                v = np.maximum(sidx[s0:s0 + kB * BLK] - HALF, 0)
                idx_all[:, cB[bb]:cB[bb] + kB * 8] = _wrap_idx16(v)

        eT = np.zeros((De, SLOTS), np.float32)
        real = eperm >= 0
        eT[:, real] = edge_attr[eperm[real]].T
        dloc_cols = dloc.reshape(TOT, BLK).T.copy()   # [128, TOT]
        # host-built transposed one-hots: ohrT[d, slot] = (dloc[slot] == d)
        ohrT_all = (dloc[None, :] == np.arange(128, dtype=np.float32)[:, None])

        xT = np.zeros((Din + 1, NPP), np.float32)
        xT[:Din, :NP] = x[c * NP:(c + 1) * NP].T
        xT[Din, :NP] = 1.0
        W1a = np.concatenate([W1, b1[None, :]], axis=0).astype(np.float32)

        bc = np.full(NPP, -1.0, np.float32)
        bc[:NP] = batch[c * NP:(c + 1) * NP].astype(np.float32)
        bcols = bc.reshape(NBLK, BLK).T.copy()

        m = {
            "xT": xT, "W1a": W1a,
            "idx": idx_all,
            "dloc": dloc_cols.astype(BF16),
            "ohrt": ohrT_all.astype(FP8),
            "eT": eT.astype(FP8),
            "iota": np.tile(np.arange(128, dtype=np.float32)[None, :],
                            (128, 1)).astype(BF16),
            "iotac": np.arange(128, dtype=np.float32)[:, None].copy(),
            "ident": np.eye(128, dtype=np.float32),
            "ones_col": np.ones((128, 1), np.float32),
            "ones_row": np.ones((1, 128), np.float32),
            "onesr_bf": np.ones((1, 128), BF16),
            "bcols": bcols.astype(np.float32),
            "invc": inv_cnt[:, None].astype(np.float32),
            "W2": W2.astype(np.float32),
            "b2b": np.tile(b2[None, :], (128, 1)).astype(np.float32),
            "W3": W3.astype(np.float32),
            "b3c": np.tile(b3[None, :], (128, 1)).astype(np.float32),
        }
        for li, (Wf, bf, Ws, bs, gw, gb) in enumerate(layer_params, start=1):
            # F halves negated so one Exp yields e^-F | e^S
            m[f"wdst{li}"] = np.concatenate([-Wf[:D], Ws[:D]], axis=1).astype(BF16)
            m[f"wsrc{li}"] = np.concatenate([-Wf[D:2 * D], Ws[D:2 * D]], axis=1).astype(BF16)
            m[f"bdst{li}"] = np.concatenate([-bf, bs])[None, :].astype(np.float32)
            m[f"wefs{li}"] = np.concatenate([-Wf[2 * D:], Ws[2 * D:]], axis=1).astype(BF16)
            m[f"gw{li}"] = gw[:, None].astype(np.float32)
            m[f"gb{li}"] = gb[:, None].astype(np.float32)
        in_maps.append(m)

    return sched, in_maps


def build_nc(sched):
    D = sched["D"]
    De = sched["De"]
    G = sched["G"]
    N = sched["N"]
    NPP = sched["NPP"]
    NBLK = sched["NBLK"]
    HALF = sched["HALF"]
    TOT = sched["TOT"]
    kAs, kBs = sched["kAs"], sched["kBs"]
    bt0 = sched["bt0"]
    KT, KTA, KTB = sched["KT"], sched["KTA"], sched["KTB"]
    cA, cB = sched["cA"], sched["cB"]
    ICOLS = sched["ICOLS"]
    SLOTS = TOT * BLK
    Din1 = sched["Din"] + 1
    NPr = sched["NP"]
    n_layers = 3
    LOCAL_LN = bool(os.environ.get("CG_LOCAL_LN"))
    inv_ND = (1.0 / (float(NPr) * float(D))) if LOCAL_LN \
        else (1.0 / (float(N) * float(D)))

    import time as _time
    _t0 = _time.time()
    nc = bacc.Bacc("TRN2", target_bir_lowering=False, debug=False,
                   num_devices=NC_CORES)
    rg = [list(range(NC_CORES))]

    ins = {}

    def inp(name, shape, dt):
        ins[name] = nc.dram_tensor(name, list(shape), dt, kind="ExternalInput")
        return ins[name]

    xT_d = inp("xT", (Din1, NPP), F32)
    W1a_d = inp("W1a", (Din1, D), F32)
    idx_d = inp("idx", (128, ICOLS), I16)
    dloc_d = inp("dloc", (128, TOT), BF)
    ohrt_d = inp("ohrt", (128, SLOTS), F8)
    eT_d = inp("eT", (De, SLOTS), F8)
    iota_d = inp("iota", (128, 128), BF)
    iotac_d = inp("iotac", (128, 1), F32)
    ident_d = inp("ident", (128, 128), F32)
    onesc_d = inp("ones_col", (128, 1), F32)
    onesr_d = inp("ones_row", (1, 128), F32)
    onesrbf_d = inp("onesr_bf", (1, 128), BF)
    bcols_d = inp("bcols", (128, NBLK), F32)
    invc_d = inp("invc", (128, 1), F32)
    W2_d = inp("W2", (D, 16), F32)
    b2b_d = inp("b2b", (128, 16), F32)
    W3_d = inp("W3", (16, 1), F32)
    b3c_d = inp("b3c", (128, 1), F32)
    for li in range(1, n_layers + 1):
        inp(f"wdst{li}", (D, 2 * D), BF)
        inp(f"wsrc{li}", (D, 2 * D), BF)
        inp(f"bdst{li}", (1, 2 * D), F32)
        inp(f"wefs{li}", (De, 2 * D), BF)
        inp(f"gw{li}", (128, 1), F32)
        inp(f"gb{li}", (128, 1), F32)

    out_d = nc.dram_tensor("out", [G, 1], F32, kind="ExternalOutput")

    ag_in = [nc.dram_tensor(f"ag_in{li}", [NPP, 2 * D], BF, kind="Internal")
             for li in range(n_layers)]
    table = [nc.dram_tensor(f"table{li}", [NC_CORES * NPP, 2 * D], BF,
                            kind="Internal", addr_space="Shared")
             for li in range(n_layers)]
    st_in = [nc.dram_tensor(f"st_in{li}", [1, 2], F32, kind="Internal")
             for li in range(n_layers)]
    st_out = [nc.dram_tensor(f"st_out{li}", [1, 2], F32, kind="Internal",
                             addr_space="Shared")
              for li in range(n_layers)]
    pool_in = nc.dram_tensor("pool_in", [G, D], F32, kind="Internal")
    pool_out = nc.dram_tensor("pool_out", [G, D], F32, kind="Internal",
                              addr_space="Shared")

    with tile.TileContext(nc) as tc:
        with (
            tc.tile_pool(name="const", bufs=1) as cpool,
            tc.tile_pool(name="state", bufs=1) as spool,
            tc.tile_pool(name="gath", bufs=5) as gpool,
            tc.tile_pool(name="et", bufs=2) as epool,
            tc.tile_pool(name="ring", bufs=3) as rpool,
            tc.tile_pool(name="oh", bufs=4) as opool,
            tc.tile_pool(name="work", bufs=2) as wpool,
            tc.tile_pool(name="psE", bufs=2, space="PSUM") as psE_p,
            tc.tile_pool(name="psA", bufs=2, space="PSUM") as psA_p,
            tc.tile_pool(name="psT", bufs=2, space="PSUM") as psT_p,
        ):
            nc.gpsimd.load_library(_mlp_lib)

            def load_const(d, shape, dt, tag):
                t = cpool.tile(list(shape), dt, tag=tag)
                nc.sync.dma_start(t[:], d[:])
                return t

            iota = load_const(iota_d, (128, 128), BF, "iota")
            iotac = load_const(iotac_d, (128, 1), F32, "iotac")
            ident = load_const(ident_d, (128, 128), F32, "ident")
            identb = cpool.tile([128, 128], BF, tag="identb")
            nc.vector.tensor_copy(out=identb[:], in_=ident[:])
            ones_col = load_const(onesc_d, (128, 1), F32, "onesc")
            ones_row = load_const(onesr_d, (1, 128), F32, "onesr")
            onesr_bf = load_const(onesrbf_d, (1, 128), BF, "onesrbf")
            dloc = load_const(dloc_d, (128, TOT), BF, "dloc")
            idx = load_const(idx_d, (128, ICOLS), I16, "idx")
            bcols = load_const(bcols_d, (128, NBLK), F32, "bcols")
            invc = load_const(invc_d, (128, 1), F32, "invc")
            W2s = load_const(W2_d, (D, 16), F32, "W2")
            b2b = load_const(b2b_d, (128, 16), F32, "b2b")
            W3s = load_const(W3_d, (16, 1), F32, "W3")
            b3c = load_const(b3c_d, (128, 1), F32, "b3c")
            wefs = [load_const(ins[f"wefs{li}"], (De, 2 * D), BF, f"wefs{li}")
                    for li in range(1, n_layers + 1)]
            wsrc = [load_const(ins[f"wsrc{li}"], (D, 2 * D), BF, f"wsrc{li}")
                    for li in range(1, n_layers + 1)]
            wdst = [load_const(ins[f"wdst{li}"], (D, 2 * D), BF, f"wdst{li}")
                    for li in range(1, n_layers + 1)]
            bdst = [load_const(ins[f"bdst{li}"], (1, 2 * D), F32, f"bdst{li}")
                    for li in range(1, n_layers + 1)]
            gw = [load_const(ins[f"gw{li}"], (128, 1), F32, f"gw{li}")
                  for li in range(1, n_layers + 1)]
            gb = [load_const(ins[f"gb{li}"], (128, 1), F32, f"gb{li}")
                  for li in range(1, n_layers + 1)]
            W1a = load_const(W1a_d, (Din1, D), F32, "W1a")

            hT = spool.tile([128, NPP], F32, tag="hT")
            dstp = spool.tile([128, NBLK, 2 * D], BF, tag="dstp")

            # ---- FC1 ----
            for b in range(NBLK):
                xTb = wpool.tile([Din1, BLK], F32, tag="xTb")
                nc.sync.dma_start(xTb[:], xT_d[:, b * BLK:(b + 1) * BLK])
                ps = psT_p.tile([128, BLK], F32, tag="t", space="PSUM")
                nc.tensor.matmul(ps[:], lhsT=W1a[:], rhs=xTb[:],
                                 start=True, stop=True)
                nc.vector.tensor_copy(out=hT[:, b * BLK:(b + 1) * BLK], in_=ps[:])

            for li in range(n_layers):
                # ---- node-side prep: src projections (-> AllGather table)
                #      and dst projections (-> SBUF) ----
                for b in range(NBLK):
                    hsl = hT[:, b * BLK:(b + 1) * BLK]
                    hslb = wpool.tile([128, BLK], BF, tag="hslb")
                    nc.scalar.activation(hslb[:], hsl, AF.Copy)
                    psS = psE_p.tile([128, 2 * D], F32, tag="pe", space="PSUM")
                    nc.tensor.matmul(psS[:], lhsT=hslb[:], rhs=wsrc[li][:],
                                     start=True, stop=True)
                    stS = wpool.tile([128, 2 * D], BF, tag="stS")
                    nc.scalar.activation(stS[:], psS[:], AF.Exp)
                    nc.sync.dma_start(ag_in[li][b * BLK:(b + 1) * BLK, :], stS[:])
                    psD = psE_p.tile([128, 2 * D], F32, tag="pe", space="PSUM")
                    nc.tensor.matmul(psD[:], lhsT=hslb[:], rhs=wdst[li][:],
                                     start=True, stop=False)
                    nc.tensor.matmul(psD[:], lhsT=ones_row[:], rhs=bdst[li][:],
                                     start=False, stop=True)
                    nc.vector.tensor_copy(out=dstp[:, b, :], in_=psD[:])
                nc.gpsimd.collective_compute(
                    "AllGather", OP.bypass, replica_groups=rg,
                    ins=[ag_in[li][:]], outs=[table[li][:]],
                )

                # ---- edge phase ----
                tabA = table[li][0:HALF, :]
                tabB = table[li][HALF:2 * HALF, :]

                GRP = 4
                pend = None   # (uv, ohr_g, b, kt)

                def scatter_block(p):
                    uv_s, ohr_s, b, kt = p
                    agg = psA_p.tile([128, BLK], F32, tag="agg",
                                     name=f"agg{b}", space="PSUM")
                    for j in range(kt):
                        nc.tensor.matmul(
                            agg[:], lhsT=uv_s[:, j, :D], rhs=ohr_s[:, j, :],
                            start=(j == 0), stop=(j == kt - 1))
                    nc.vector.tensor_tensor(
                        out=hT[:, b * BLK:(b + 1) * BLK],
                        in0=hT[:, b * BLK:(b + 1) * BLK],
                        in1=agg[:], op=OP.add)

                for b in range(NBLK):
                    kA, kB = kAs[b], kBs[b]
                    kt = kA + kB
                    tA = bt0[b]
                    s0 = tA * BLK
                    L = kt * BLK
                    # streams for this block
                    ech = epool.tile([De, KT * BLK], F8, tag="ech")
                    nc.sync.dma_start(ech[:, :L], eT_d[:, s0:s0 + L])
                    chA = chB = None
                    if kA:
                        chA = gpool.tile([128, KTA, 2 * D], BF, tag="gA")
                        nc.gpsimd.dma_gather(
                            chA[:, :kA, :], tabA, idx[:, cA[b]:cA[b] + kA * 8],
                            kA * BLK, kA * BLK, 2 * D)
                    if kB:
                        chB = gpool.tile([128, KTB, 2 * D], BF, tag="gB")
                        nc.gpsimd.dma_gather(
                            chB[:, :kB, :], tabB, idx[:, cB[b]:cB[b] + kB * 8],
                            kB * BLK, kB * BLK, 2 * D)
                    # one-hots (batched per block)
                    ohr_g = opool.tile([128, KT, 128], BF, tag="ohr")
                    nc.vector.tensor_tensor(
                        out=ohr_g[:, :kt, :],
                        in0=iota[:].rearrange("p (o f) -> p o f", o=1)
                            .to_broadcast([128, kt, 128]),
                        in1=dloc[:, tA:tA + kt]
                            .rearrange("p (t o) -> p t o", o=1)
                            .to_broadcast([128, kt, 128]),
                        op=OP.is_equal)
                    ohrT_g = opool.tile([128, KT, 128], F8, tag="ohrT")
                    nc.sync.dma_start(
                        ohrT_g[:, :kt, :].rearrange("p t f -> p (t f)"),
                        ohrt_d[:, s0:s0 + L])
                    # psE groups: eproj + dst-inject, then exp
                    uv = rpool.tile([128, KT, 2 * D], BF, tag="uv")
                    for g0 in range(0, kt, GRP):
                        gw_ = min(GRP, kt - g0)
                        psE = psE_p.tile([128, GRP * 2 * D], F32, tag="pe",
                                         space="PSUM")
                        for j in range(gw_):
                            t = g0 + j
                            sl = slice(j * 2 * D, (j + 1) * 2 * D)
                            nc.tensor.matmul(
                                psE[:, sl], lhsT=ech[:, (t) * BLK:(t + 1) * BLK],
                                rhs=wefs[li][:], start=True, stop=False)
                            nc.tensor.matmul(
                                psE[:, sl], lhsT=ohrT_g[:, t, :],
                                rhs=dstp[:, b, :], start=False, stop=True)
                        nc.scalar.activation(uv[:, g0:g0 + gw_, :],
                                             psE[:, :gw_ * 2 * D], AF.Exp)
                    # inject exponentiated src factors (batched DVE mults)
                    with nc.allow_low_precision(reason="bf16 edge pipeline"):
                        if kA:
                            nc.vector.tensor_tensor(
                                out=uv[:, :kA, :], in0=uv[:, :kA, :],
                                in1=chA[:, :kA, :], op=OP.mult)
                        if kB:
                            nc.vector.tensor_tensor(
                                out=uv[:, kA:kt, :], in0=uv[:, kA:kt, :],
                                in1=chB[:, :kB, :], op=OP.mult)
                    # sp = ln(1+v); sigma = e^-ln(1+u); m = sigma*sp
                    nc.scalar.activation(uv[:, :kt, :], uv[:, :kt, :],
                                         AF.Ln, bias=1.0)
                    nc.scalar.activation(uv[:, :kt, :D], uv[:, :kt, :D],
                                         AF.Exp, scale=-1.0)
                    with nc.allow_low_precision(reason="bf16 edge pipeline"):
                        nc.vector.tensor_tensor(out=uv[:, :kt, :D],
                                                in0=uv[:, :kt, :D],
                                                in1=uv[:, :kt, D:], op=OP.mult)
                    if pend is not None:
                        scatter_block(pend)
                    pend = (uv, ohr_g, b, kt)
                if pend is not None:
                    scatter_block(pend)

                # ---- graph LayerNorm + relu ----
                stats = spool.tile([128, 2], F32, tag="stats")
                nc.vector.reduce_sum(stats[:, 0:1], hT[:, :NPr],
                                     axis=mybir.AxisListType.X)
                sq = wpool.tile([128, 512], F32, tag="sq")
                sqacc = spool.tile([128, _ceil_div(NPr, 512)], F32, tag="sqacc")
                for k in range(_ceil_div(NPr, 512)):
                    k0, k1 = k * 512, min(NPr, k * 512 + 512)
                    nc.scalar.activation(sq[:, :k1 - k0], hT[:, k0:k1], AF.Square)
                    nc.vector.reduce_sum(sqacc[:, k:k + 1], sq[:, :k1 - k0],
                                         axis=mybir.AxisListType.X)
                nc.vector.reduce_sum(stats[:, 1:2], sqacc[:],
                                     axis=mybir.AxisListType.X)
                psR = psT_p.tile([128, BLK], F32, tag="t", space="PSUM")
                nc.tensor.matmul(psR[:1, :2], lhsT=ones_col[:], rhs=stats[:],
                                 start=True, stop=True)
                stl = wpool.tile([1, 2], F32, tag="stl")
                nc.vector.tensor_copy(out=stl[:], in_=psR[:1, :2])
                if LOCAL_LN:
                    stg = stl
                else:
                    nc.sync.dma_start(st_in[li][:], stl[:])
                    nc.gpsimd.collective_compute(
                        "AllReduce", OP.add, replica_groups=rg,
                        ins=[st_in[li][:]], outs=[st_out[li][:]])
                    stg = wpool.tile([1, 2], F32, tag="stl")
                    nc.sync.dma_start(stg[:], st_out[li][:])
                psBc = psT_p.tile([128, BLK], F32, tag="t", space="PSUM")
                nc.tensor.matmul(psBc[:, :2], lhsT=ones_row[:], rhs=stg[:],
                                 start=True, stop=True)
                stb = wpool.tile([128, 2], F32, tag="stb")
                nc.vector.tensor_scalar(out=stb[:], in0=psBc[:, :2],
                                        scalar1=inv_ND, scalar2=None, op0=OP.mult)
                mean = stb[:, 0:1]
                ex2 = stb[:, 1:2]
                var = wpool.tile([128, 1], F32, tag="v1")
                nc.vector.tensor_tensor(out=var[:], in0=mean, in1=mean, op=OP.mult)
                nc.vector.tensor_tensor(out=var[:], in0=ex2, in1=var[:],
                                        op=OP.subtract)
                std = wpool.tile([128, 1], F32, tag="v2")
                nc.scalar.activation(std[:], var[:], AF.Sqrt)
                nc.vector.tensor_scalar(out=std[:], in0=std[:], scalar1=float(EPS),
                                        scalar2=None, op0=OP.add)
                inv = wpool.tile([128, 1], F32, tag="v3")
                nc.vector.reciprocal(inv[:], std[:])
                scale_c = wpool.tile([128, 1], F32, tag="v4")
                nc.vector.tensor_tensor(out=scale_c[:], in0=inv[:], in1=gw[li][:],
                                        op=OP.mult)
                bias_c = wpool.tile([128, 1], F32, tag="v5")
                nc.vector.tensor_tensor(out=bias_c[:], in0=mean, in1=scale_c[:],
                                        op=OP.mult)
                nc.vector.tensor_tensor(out=bias_c[:], in0=gb[li][:], in1=bias_c[:],
                                        op=OP.subtract)
                for k in range(_ceil_div(NPP, 512)):
                    k0, k1 = k * 512, min(NPP, k * 512 + 512)
                    tmp = wpool.tile([128, 512], F32, tag="lnt")
                    nc.vector.tensor_scalar(
                        out=tmp[:, :k1 - k0], in0=hT[:, k0:k1],
                        scalar1=scale_c[:], scalar2=bias_c[:],
                        op0=OP.mult, op1=OP.add)
                    nc.scalar.activation(hT[:, k0:k1], tmp[:, :k1 - k0], AF.Relu)

            # ---- pool + MLP ----
            pool_ps = psA_p.tile([128, D], F32, tag="agg", name="pool", space="PSUM")
            for b in range(NBLK):
                psTt = psT_p.tile([128, BLK], F32, tag="t", space="PSUM")
                nc.tensor.transpose(psTt[:], hT[:, b * BLK:(b + 1) * BLK], ident[:])
                h3 = wpool.tile([128, D], BF, tag="h3")
                nc.vector.tensor_copy(out=h3[:], in_=psTt[:])
                ohg = wpool.tile([128, 128], BF, tag="ohg")
                nc.vector.tensor_scalar(
                    out=ohg[:], in0=iota[:], scalar1=bcols[:, b:b + 1],
                    scalar2=None, op0=OP.is_equal)
                nc.tensor.matmul(pool_ps[:], lhsT=ohg[:], rhs=h3[:],
                                 start=(b == 0), stop=(b == NBLK - 1))
            pool_sb = wpool.tile([G, D], F32, tag="poolsb")
            nc.vector.tensor_copy(out=pool_sb[:], in_=pool_ps[:])
            nc.sync.dma_start(pool_in[:], pool_sb[:])
            nc.gpsimd.collective_compute(
                "AllReduce", OP.add, replica_groups=rg,
                ins=[pool_in[:]], outs=[pool_out[:]])
            hg = wpool.tile([G, D], F32, tag="poolsb")
            nc.sync.dma_start(hg[:], pool_out[:])
            nc.vector.tensor_scalar(out=hg[:], in0=hg[:], scalar1=invc[:],
                                    scalar2=None, op0=OP.mult)
            psT2 = psT_p.tile([128, BLK], F32, tag="t", space="PSUM")
            nc.tensor.transpose(psT2[:], hg[:], ident[:])
            hgT = wpool.tile([D, G], F32, tag="hgT")
            nc.vector.tensor_copy(out=hgT[:], in_=psT2[:])
            ps2 = psT_p.tile([128, BLK], F32, tag="t", space="PSUM")
            nc.tensor.matmul(ps2[:, :16], lhsT=hgT[:], rhs=W2s[:],
                             start=True, stop=True)
            h2 = wpool.tile([G, 16], F32, tag="h2")
            nc.vector.tensor_tensor(out=h2[:], in0=ps2[:, :16], in1=b2b[:],
                                    op=OP.add)
            nc.scalar.activation(h2[:], h2[:], AF.Relu)
            psT3 = psT_p.tile([128, BLK], F32, tag="t", space="PSUM")
            nc.tensor.transpose(psT3[:16, :], h2[:], ident[:])
            h2T = wpool.tile([16, G], F32, tag="h2T")
            nc.vector.tensor_copy(out=h2T[:], in_=psT3[:16, :])
            ps3 = psT_p.tile([128, BLK], F32, tag="t", space="PSUM")
            nc.tensor.matmul(ps3[:, :1], lhsT=h2T[:], rhs=W3s[:],
                             start=True, stop=True)
            outsb = wpool.tile([G, 1], F32, tag="outsb")
            nc.vector.tensor_tensor(out=outsb[:], in0=ps3[:, :1], in1=b3c[:],
                                    op=OP.add)
            nc.sync.dma_start(out_d[:], outsb[:])

    _t1 = _time.time()
    print(f"[build_nc v2] trace: {_t1 - _t0:.1f}s, instrs: "
          f"{sum(len(bb.instructions) for bb in nc.main_func.blocks)}", flush=True)
    nc.compile()
    print(f"[build_nc v2] bass compile: {_time.time() - _t1:.1f}s", flush=True)
    return nc


def kernel(x, edge_index, edge_attr, batch,
           W1, b1,
           Wf1, bf1, Ws1, bs1, g1w, g1b,
           Wf2, bf2, Ws2, bs2, g2w, g2b,
           Wf3, bf3, Ws3, bs3, g3w, g3b,
           W2, b2, W3, b3):
    layer_params = [
        (Wf1, bf1, Ws1, bs1, g1w, g1b),
        (Wf2, bf2, Ws2, bs2, g2w, g2b),
        (Wf3, bf3, Ws3, bs3, g3w, g3b),
    ]
    sched, in_maps = host_prep(np.asarray(x), np.asarray(edge_index),
                               np.asarray(edge_attr), np.asarray(batch),
                               np.asarray(W1), np.asarray(b1), layer_params,
                               np.asarray(W2), np.asarray(b2),
                               np.asarray(W3), np.asarray(b3))
    import time as _time
    _tp = _time.time()
    nc = build_nc(sched)
    print(f"[kernel v2] build done {_time.time() - _tp:.1f}s", flush=True)
    _tr = _time.time()
    res = run_bass_kernel_spmd(nc, in_maps, core_ids=list(range(NC_CORES)),
                               tmpdir=os.environ.get("BASS_TMPDIR") or None)
    print(f"[kernel v2] run (incl neff compile) {_time.time() - _tr:.1f}s",
          flush=True)
    if res.exec_time_ns is not None:
        print(f"HW exec time: {res.exec_time_ns} ns", flush=True)
    return np.asarray(res.results[0]["out"], np.float32)
